# revision 1
# baseline (speedup 1.0000x reference)
"""Binary 3-layer CNN (sign activations + sign weights) on 8 NeuronCores.

Strategy: pure data parallel — 64 images -> 8 cores x 8 images.
Per core: 2 batches of 4 images; SBUF partition layout [128 = (4 img, 32 ch), pix].
Each 3x3 conv = 9 PSUM-accumulating matmuls with block-diagonal stationary
weights (4 identical 32x32 blocks) and free-dim-shifted rhs APs (dy*Wpad+dx),
so there is no im2col data movement. All matmul operands are exactly +-1/0 ->
bf16 with fp32 PSUM accumulation is numerically exact. sign() is applied by
ScalarE during PSUM->SBUF eviction. Layers staged through DRAM scratch in a
zero-padded layout (pad rows AND pad cols pre-zeroed in DRAM once) so conv
padding is baked in and SBUF tiles are single-producer.
"""

import numpy as np
import ml_dtypes

import concourse.bass as bass
import concourse.mybir as mybir
import concourse.tile as tile
from concourse import bacc
from concourse.bass_utils import run_bass_kernel_spmd

BF16 = mybir.dt.bfloat16
F32 = mybir.dt.float32
AF = mybir.ActivationFunctionType

N_CORES = 8
IMG_PER_CORE = 8
B = 4          # images per partition-batch
H = W = 256
WP = 258       # padded width (1 col pad each side)
HP = 258       # padded height
R = 64         # strip rows (stages A/B)
RC = 32        # strip rows (stage C)
NB = IMG_PER_CORE // B  # batches per core


def _conv_strip(nc, pspool, lhsT_taps, hin, dst_evict, rows):
    """rows output rows; hin is [*, rows+2, WP] (padded); evict 4 psum tiles."""
    mparts = lhsT_taps[0].shape[-1]
    for r0 in range(0, rows, 8):  # groups of 4 psum tiles (2 rows each)
        pss = [pspool.tile([mparts, 2, 256], F32, tag=f"ps{p}", name=f"ps{p}")
               for p in range(4)]
        for tap in range(9):
            dy, dx = tap // 3, tap % 3
            for p in range(4):
                r = r0 + 2 * p
                nc.tensor.matmul(
                    pss[p][:, :, :],
                    lhsT_taps[tap],
                    hin[:, r + dy:r + dy + 2, dx:dx + 256],
                    start=(tap == 0), stop=(tap == 8))
        dst_evict(pss, r0)


def _build_program(stages=('0','A','B','C')):
    nc = bacc.Bacc("TRN2", target_bir_lowering=False, debug=False)

    x_in = nc.dram_tensor("x", [IMG_PER_CORE, H, W], F32, kind="ExternalInput")
    s0_in = nc.dram_tensor("s0", [12, 3, 128], BF16, kind="ExternalInput")
    s1_in = nc.dram_tensor("s1", [128, 9, 128], BF16, kind="ExternalInput")
    s2_in = nc.dram_tensor("s2", [128, 9, B], BF16, kind="ExternalInput")
    out_d = nc.dram_tensor("out", [IMG_PER_CORE, H, W], F32, kind="ExternalOutput")

    xs_d = nc.dram_tensor("xs", [IMG_PER_CORE, HP, WP], BF16)
    h0_d = nc.dram_tensor("h0", [NB, 128, HP, WP], BF16)
    h1_d = nc.dram_tensor("h1", [NB, 128, HP, WP], BF16)

    with tile.TileContext(nc) as tc:
        with (
            tc.tile_pool(name="const", bufs=1) as cpool,
            tc.tile_pool(name="xprep", bufs=4) as xpool,
            tc.tile_pool(name="ain", bufs=2) as apool,
            tc.tile_pool(name="aout", bufs=2) as opool,
            tc.tile_pool(name="cout", bufs=1) as cpool2,
            tc.tile_pool(name="psum", bufs=2, space="PSUM") as pspool,
        ):
            # --- constants: stationary weights + a zero tile ---
            # s0 replicated into 4 row groups (base partitions 0/32/64/96)
            # so 4 psum tiles' conv0 matmuls run concurrently via row tiling
            s0t = cpool.tile([108, 3, 128], BF16, tag="s0")
            for p in range(4):
                nc.sync.dma_start(out=s0t[32 * p:32 * p + 12, :, :],
                                  in_=s0_in[:, :, :])
            s1t = cpool.tile([128, 9, 128], BF16, tag="s1")
            nc.sync.dma_start(out=s1t[:, :, :], in_=s1_in[:, :, :])
            s2t = cpool.tile([128, 9, B], BF16, tag="s2")
            nc.sync.dma_start(out=s2t[:, :, :], in_=s2_in[:, :, :])
            zt = cpool.tile([128, WP], BF16, tag="zt")
            nc.gpsimd.memset(zt[:, :], 0.0)

            # --- pre-zero DRAM pad rows (contiguous); col pads are baked
            # into the SBUF tiles below ---
            for img in range(IMG_PER_CORE):
                nc.scalar.dma_start(out=xs_d[img, 0:1, :], in_=zt[0:1, :])
                nc.scalar.dma_start(out=xs_d[img, HP - 1:HP, :], in_=zt[0:1, :])
            for b in range(NB):
                for hd in (h0_d, h1_d):
                    nc.scalar.dma_start(out=hd[b, :, 0, :], in_=zt[:, :])
                    nc.scalar.dma_start(out=hd[b, :, HP - 1, :], in_=zt[:, :])

            # --- stage 0: sign(x) -> padded bf16 planes in DRAM ---
            for img in range(IMG_PER_CORE if '0' in stages else 0):
                for rb in range(H // 128):
                    xf = xpool.tile([128, W], F32, tag="xf")
                    nc.sync.dma_start(
                        out=xf[:, :], in_=x_in[img, rb * 128:(rb + 1) * 128, :])
                    xp = xpool.tile([128, WP], BF16, tag="xp")
                    nc.scalar.activation(xp[:, 1:W + 1], xf[:, :], AF.Sign)
                    nc.vector.memset(xp[:, 0:1], 0.0)
                    nc.vector.memset(xp[:, WP - 1:WP], 0.0)
                    nc.scalar.dma_start(
                        out=xs_d[img, rb * 128 + 1:(rb + 1) * 128 + 1, :],
                        in_=xp[:, :])

            for b in range(NB):
                # ---- stage A: conv0 (1 -> 32ch), dy-in-K: K=12, M=128,
                # 4x row tiling: input replicated to partition groups
                # 0/32/64/96; the 4 psum tiles' matmuls occupy distinct
                # 32-row strips of the PE array and run concurrently ----
                for s in range(H // R if 'A' in stages else 0):
                    xt = apool.tile([108, R, WP], BF16, tag="lin")
                    for p in range(4):
                        for dy in range(3):
                            nc.sync.dma_start(
                                out=xt[32 * p + dy * B:32 * p + (dy + 1) * B,
                                       :, :],
                                in_=xs_d[b * B:(b + 1) * B,
                                         s * R + dy:s * R + dy + R, :])
                    ht = opool.tile([128, R, WP], BF16, tag="a_out")
                    nc.vector.memset(ht[:, :, 0:1], 0.0)
                    nc.vector.memset(ht[:, :, WP - 1:WP], 0.0)
                    for r0 in range(0, R, 8):
                        pss = [pspool.tile([128, 2, 256], F32,
                                           tag=f"ps{p}", name=f"ps{p}")
                               for p in range(4)]
                        for dx in range(3):
                            for p in range(4):
                                r = r0 + 2 * p
                                nc.tensor.matmul(
                                    pss[p][:, :, :],
                                    s0t[32 * p:32 * p + 12, dx, :],
                                    xt[32 * p:32 * p + 12, r:r + 2,
                                       dx:dx + 256],
                                    start=(dx == 0), stop=(dx == 2),
                                    tile_position=(32 * p, 0))
                        for p in range(4):
                            r = r0 + 2 * p
                            nc.scalar.activation(
                                ht[:, r:r + 2, 1:W + 1], pss[p][:, :, :], AF.Sign)
                    nc.scalar.dma_start(
                        out=h0_d[b, :, s * R + 1:s * R + R + 1, :],
                        in_=ht[:, :, :])

                # ---- stage B: conv1 (32 -> 32ch), K=128, M=128 ----
                for s in range(H // R if 'B' in stages else 0):
                    hin = apool.tile([128, R + 2, WP], BF16, tag="lin")
                    nc.sync.dma_start(
                        out=hin[:, :, :], in_=h0_d[b, :, s * R:s * R + R + 2, :])
                    ht = opool.tile([128, R, WP], BF16, tag="a_out")
                    nc.vector.memset(ht[:, :, 0:1], 0.0)
                    nc.vector.memset(ht[:, :, WP - 1:WP], 0.0)

                    def evict_b(pss, r0, ht=ht):
                        for p in range(4):
                            r = r0 + 2 * p
                            nc.scalar.activation(
                                ht[:, r:r + 2, 1:W + 1], pss[p][:, :, :], AF.Sign)

                    _conv_strip(nc, pspool,
                                [s1t[:, t, :] for t in range(9)], hin, evict_b, R)
                    nc.scalar.dma_start(
                        out=h1_d[b, :, s * R + 1:s * R + R + 1, :],
                        in_=ht[:, :, :])

                # ---- stage C: conv2 (32 -> 1ch), K=128, M=4, 4x col-tiling ----
                # 4 psum row-pairs go to col groups 0/32/64/96 of the SAME
                # psum tile; the 4 matmuls per tap run concurrently on
                # distinct 32-col strips of the PE array.
                for s in range(H // RC if 'C' in stages else 0):
                    hin = apool.tile([128, RC + 2, WP], BF16, tag="lin")
                    nc.sync.dma_start(
                        out=hin[:, :, :], in_=h1_d[b, :, s * RC:s * RC + RC + 2, :])
                    ot = cpool2.tile([B, RC, W], F32, tag="c_out")
                    for r0 in range(0, RC, 8):
                        ps = pspool.tile([128, 2, 256], F32, tag="ps0", name="psc")
                        for tap in range(9):
                            dy, dx = tap // 3, tap % 3
                            for p in range(4):
                                r = r0 + 2 * p
                                nc.tensor.matmul(
                                    ps[32 * p:32 * p + B, :, :],
                                    s2t[:, tap, :],
                                    hin[:, r + dy:r + dy + 2, dx:dx + 256],
                                    start=(tap == 0), stop=(tap == 8),
                                    tile_position=(0, 32 * p))
                        for p in range(4):
                            r = r0 + 2 * p
                            nc.vector.tensor_copy(
                                ot[:, r:r + 2, :], ps[32 * p:32 * p + B, :, :])
                    for g in range(B):
                        nc.scalar.dma_start(
                            out=out_d[b * B + g, s * RC:s * RC + RC, :],
                            in_=ot[g:g + 1, :, :])
    nc.compile()
    return nc


def _host_weights(w0, w1, w2):
    """Build bf16 block-diag stationary matrices. tap index = dy*3+dx."""
    sg = lambda w: np.sign(w).astype(ml_dtypes.bfloat16)
    w0s, w1s, w2s = sg(w0), sg(w1), sg(w2)   # [32,1,3,3],[32,32,3,3],[1,32,3,3]
    s0 = np.zeros((12, 3, 128), ml_dtypes.bfloat16)
    s1 = np.zeros((128, 9, 128), ml_dtypes.bfloat16)
    s2 = np.zeros((128, 9, B), ml_dtypes.bfloat16)
    for g in range(B):
        for dy in range(3):
            for dx in range(3):
                t = dy * 3 + dx
                # lhsT[k, m]: out[m] += sum_k lhsT[k,m]*rhs[k]
                # s0 [dy*4+g, dx, g*32+co]
                s0[dy * B + g, dx, g * 32:(g + 1) * 32] = w0s[:, 0, dy, dx]
                s1[g * 32:(g + 1) * 32, t, g * 32:(g + 1) * 32] = \
                    w1s[:, :, dy, dx].T  # [ci, co]
                s2[g * 32:(g + 1) * 32, t, g] = w2s[0, :, dy, dx]
    return s0, s1, s2


_NC_CACHE = {}


def kernel(x, w0, w1, w2):
    if "nc" not in _NC_CACHE:
        _NC_CACHE["nc"] = _build_program()
    nc = _NC_CACHE["nc"]
    s0, s1, s2 = _host_weights(np.asarray(w0), np.asarray(w1), np.asarray(w2))
    x = np.asarray(x, np.float32).reshape(64, H, W)
    in_maps = [
        {"x": np.ascontiguousarray(x[i * IMG_PER_CORE:(i + 1) * IMG_PER_CORE]),
         "s0": s0, "s1": s1, "s2": s2}
        for i in range(N_CORES)
    ]
    res = run_bass_kernel_spmd(nc, in_maps, list(range(N_CORES)))
    out = np.stack([np.asarray(res.results[i]["out"], np.float32)
                    for i in range(N_CORES)])
    return out.reshape(64, 1, H, W)



# revision 6
# speedup vs baseline: 2.0842x; 2.0842x over previous
"""Binary 3-layer CNN (sign activations + sign weights) on 8 NeuronCores.

Strategy: pure data parallel — 64 images -> 8 cores x 8 images.
Per core: 2 batches of 4 images; SBUF partition layout [128 = (4 img, 32 ch)].

This substrate has a large fixed cost per *instruction* (~0.1 ms) that is
nearly independent of instruction size, so the design minimizes instruction
count:
 - conv0: all 9 taps packed into the contraction dim (K=36 = 9 taps x 4
   images, via 9 tap-shifted DMA loads) -> ONE matmul per PSUM fill.
 - conv1/conv2: operands in fp8e4 (values +-1/0, exact) so tap-PAIRS pack
   into one DoubleRow matmul (2 K-tiles per instruction) -> 5 matmuls per
   fill instead of 9. rhs k-tile dim is a hand-built access pattern whose
   stride is the inter-tap offset (dy*WP + dx delta).
 - PSUM tiles span 4 banks [128, 2048]; each matmul writes one 512-f32 bank
   slice (ISA limit), one Sign activation evicts all 4 banks.
All matmul operands are exactly +-1/0 -> fp32 PSUM accumulation is exact.
Layers staged through padded DRAM scratch so conv padding is baked in.
"""

import numpy as np
import ml_dtypes

import concourse.bass as bass
import concourse.mybir as mybir
import concourse.tile as tile
from concourse import bacc
from concourse.bass_types import AP as RawAP
from concourse.bass_utils import run_bass_kernel_spmd

BF16 = mybir.dt.bfloat16
FP8 = mybir.dt.float8e4
F32 = mybir.dt.float32
AF = mybir.ActivationFunctionType
DR = mybir.MatmulPerfMode.DoubleRow

N_CORES = 8
IMG_PER_CORE = 8
B = 4          # images per partition-batch
H = W = 256
WP = 258       # padded width
HP = 258       # padded height
R = 64         # strip rows
NB = IMG_PER_CORE // B
TAPS = [(t // 3, t % 3) for t in range(9)]


def _conv_taps_dr(nc, ps_slice, s_t, hin, r):
    """9-tap conv into one 512-f32 psum bank slice: 4 DoubleRow + 1 single.
    hin is a padded fp8 tile [*, rows, WP]; r is the strip-local out row."""
    for pi, t0 in enumerate((0, 2, 4, 6)):
        dy0, dx0 = TAPS[t0]
        dy1, dx1 = TAPS[t0 + 1]
        delta = (dy1 - dy0) * WP + (dx1 - dx0)
        base = hin[:, r + dy0:r + dy0 + 2, dx0:dx0 + 256]
        rhs = RawAP(tensor=base.tensor,
                    ap=[[base.ap[0][0], 128], [delta, 2], [WP, 2], [1, 256]],
                    offset=base.offset)
        nc.tensor.matmul(ps_slice, s_t[:, t0:t0 + 2, :], rhs,
                         start=(pi == 0), stop=False, perf_mode=DR)
    nc.tensor.matmul(ps_slice, s_t[:, 8, :], hin[:, r + 2:r + 4, 2:WP],
                     start=False, stop=True)


def _build_program(stages=('0', 'A', 'B', 'C')):
    nc = bacc.Bacc("TRN2", target_bir_lowering=False, debug=False)

    x_in = nc.dram_tensor("x", [IMG_PER_CORE, H, W], F32, kind="ExternalInput")
    s0_in = nc.dram_tensor("s0", [36, 128], BF16, kind="ExternalInput")
    s1_in = nc.dram_tensor("s1", [128, 9, 128], FP8, kind="ExternalInput")
    s2_in = nc.dram_tensor("s2", [128, 9, 128], FP8, kind="ExternalInput")
    out_d = nc.dram_tensor("out", [IMG_PER_CORE, H, W], F32, kind="ExternalOutput")

    xs_d = nc.dram_tensor("xs", [IMG_PER_CORE, HP, WP], BF16)
    h0_d = nc.dram_tensor("h0", [NB, 128, HP, WP], FP8)
    h1_d = nc.dram_tensor("h1", [NB, 128, HP, WP], FP8)

    with tile.TileContext(nc) as tc:
        with (
            tc.tile_pool(name="const", bufs=1) as cpool,
            tc.tile_pool(name="xprep", bufs=2) as xpool,
            tc.tile_pool(name="a_in", bufs=1) as a_in_pool,
            tc.tile_pool(name="lay_in", bufs=2) as inpool,
            tc.tile_pool(name="lay_out", bufs=2) as outpool,
            tc.tile_pool(name="c_out", bufs=1) as cpool2,
            tc.tile_pool(name="psum", bufs=2, space="PSUM") as pspool,
        ):
            # --- constants ---
            s0t = cpool.tile([36, 128], BF16, tag="s0")
            nc.sync.dma_start(out=s0t[:, :], in_=s0_in[:, :])
            s1t = cpool.tile([128, 9, 128], FP8, tag="s1")
            nc.sync.dma_start(out=s1t[:, :, :], in_=s1_in[:, :, :])
            s2t = cpool.tile([128, 9, 128], FP8, tag="s2")
            nc.sync.dma_start(out=s2t[:, :, :], in_=s2_in[:, :, :])
            zt = cpool.tile([128, WP], FP8, tag="zt")
            nc.gpsimd.memset(zt[:, :], 0.0)
            ztw = cpool.tile([1, IMG_PER_CORE * WP], BF16, tag="ztw")
            nc.gpsimd.memset(ztw[:, :], 0.0)

            # --- pre-zero DRAM pad rows ---
            # xs: top and bottom padded row of every image, one DMA each
            for img in range(IMG_PER_CORE):
                nc.scalar.dma_start(out=xs_d[img, 0:1, :], in_=ztw[:, 0:WP])
                nc.scalar.dma_start(out=xs_d[img, HP - 1:HP, :],
                                    in_=ztw[:, 0:WP])
            for b in range(NB):
                for hd in (h0_d, h1_d):
                    nc.scalar.dma_start(out=hd[b, :, 0, :], in_=zt[:, :])
                    nc.scalar.dma_start(out=hd[b, :, HP - 1, :], in_=zt[:, :])

            # --- stage 0: sign(x) -> padded bf16 planes in DRAM, 4 img/tile ---
            if '0' in stages:
                for b in range(NB):
                    for rb in range(H // 128):
                        xf = xpool.tile([128, B, W], F32, tag="xf")
                        src = x_in[b * B:(b + 1) * B,
                                   rb * 128:(rb + 1) * 128, :]
                        nc.sync.dma_start(out=xf[:, :, :],
                                          in_=src.transpose([1, 0, 2]))
                        xp = xpool.tile([128, B, WP], BF16, tag="xp")
                        nc.vector.memset(xp[:, :, :], 0.0)
                        nc.scalar.activation(xp[:, :, 1:W + 1], xf[:, :, :],
                                             AF.Sign)
                        dst = xs_d[b * B:(b + 1) * B,
                                   rb * 128 + 1:(rb + 1) * 128 + 1, :]
                        nc.scalar.dma_start(out=dst.transpose([1, 0, 2]),
                                            in_=xp[:, :, :])

            for b in range(NB):
                # ---- stage A: conv0 (1 -> 32ch); K=36 (9 taps x 4 img) ----
                # x36[(t,g), r, c] = xs[g, 64s + r + dy_t, dx_t + c]
                if 'A' in stages:
                    for s in range(H // R):
                        x36 = a_in_pool.tile([36, R, W], BF16, tag="x36")
                        for t, (dy, dx) in enumerate(TAPS):
                            nc.sync.dma_start(
                                out=x36[B * t:B * (t + 1), :, :],
                                in_=xs_d[b * B:(b + 1) * B,
                                         s * R + dy:s * R + dy + R,
                                         dx:dx + 256])
                        ht = outpool.tile([128, R, WP], FP8, tag="ht")
                        nc.vector.memset(ht[:, :, :], 0.0)
                        for i in range(R // 8):
                            ps = pspool.tile([128, 2048], F32, tag="ps",
                                             name="ps")
                            for fi in range(4):
                                r = 8 * i + 2 * fi
                                nc.tensor.matmul(
                                    ps[:, fi * 512:(fi + 1) * 512],
                                    s0t[:, :], x36[:, r:r + 2, :],
                                    start=True, stop=True)
                            nc.scalar.activation(
                                ht[:, 8 * i:8 * i + 8, 1:W + 1],
                                ps[:, :].rearrange("p (a c) -> p a c", a=8),
                                AF.Sign)
                        nc.scalar.dma_start(
                            out=h0_d[b, :, s * R + 1:s * R + R + 1, :],
                            in_=ht[:, :, :])

                # ---- stage B: conv1 (32 -> 32ch); fp8 DoubleRow tap pairs ----
                if 'B' in stages:
                    for s in range(H // R):
                        hin = inpool.tile([128, R + 2, WP], FP8, tag="hin")
                        nc.sync.dma_start(
                            out=hin[:, :, :],
                            in_=h0_d[b, :, s * R:s * R + R + 2, :])
                        ht = outpool.tile([128, R, WP], FP8, tag="ht")
                        nc.vector.memset(ht[:, :, :], 0.0)
                        for i in range(R // 8):
                            ps = pspool.tile([128, 2048], F32, tag="ps",
                                             name="ps")
                            for fi in range(4):
                                _conv_taps_dr(
                                    nc, ps[:, fi * 512:(fi + 1) * 512],
                                    s1t, hin, 8 * i + 2 * fi)
                            nc.scalar.activation(
                                ht[:, 8 * i:8 * i + 8, 1:W + 1],
                                ps[:, :].rearrange("p (a c) -> p a c", a=8),
                                AF.Sign)
                        nc.scalar.dma_start(
                            out=h1_d[b, :, s * R + 1:s * R + R + 1, :],
                            in_=ht[:, :, :])

                # ---- stage C: conv2 (32 -> 1ch); M=4, fp8 DoubleRow ----
                if 'C' in stages:
                    for s in range(H // R):
                        hin = inpool.tile([128, R + 2, WP], FP8, tag="hin")
                        nc.sync.dma_start(
                            out=hin[:, :, :],
                            in_=h1_d[b, :, s * R:s * R + R + 2, :])
                        ot = cpool2.tile([B, R, W], F32, tag="ot")
                        for i in range(R // 8):
                            ps = pspool.tile([128, 2048], F32, tag="ps",
                                             name="ps")
                            for fi in range(4):
                                _conv_taps_dr(
                                    nc, ps[:, fi * 512:(fi + 1) * 512],
                                    s2t, hin, 8 * i + 2 * fi)
                            nc.vector.tensor_copy(
                                ot[:, 8 * i:8 * i + 8, :],
                                ps[0:B, :].rearrange("p (a c) -> p a c", a=8))
                        dst = out_d[b * B:(b + 1) * B, s * R:s * R + R, :]
                        nc.scalar.dma_start(out=dst, in_=ot[:, :, :])
    nc.compile()
    return nc


def _host_weights(w0, w1, w2):
    """Pack sign(w) into stationary matrices. tap index t = dy*3+dx."""
    w0s = np.sign(np.asarray(w0, np.float32))  # [32,1,3,3]
    w1s = np.sign(np.asarray(w1, np.float32))  # [32,32,3,3]
    w2s = np.sign(np.asarray(w2, np.float32))  # [1,32,3,3]
    s0 = np.zeros((36, 128), np.float32)
    s1 = np.zeros((128, 9, 128), np.float32)
    s2 = np.zeros((128, 9, 128), np.float32)
    for g in range(B):
        for t, (dy, dx) in enumerate(TAPS):
            # out[m=(g,co)] += s0[k=(t,g), m] * x36[k, pix]
            s0[t * B + g, g * 32:(g + 1) * 32] = w0s[:, 0, dy, dx]
            s1[g * 32:(g + 1) * 32, t, g * 32:(g + 1) * 32] = \
                w1s[:, :, dy, dx].T  # [ci, co]
            s2[g * 32:(g + 1) * 32, t, g] = w2s[0, :, dy, dx]
    return (s0.astype(ml_dtypes.bfloat16),
            s1.astype(ml_dtypes.float8_e4m3),
            s2.astype(ml_dtypes.float8_e4m3))


_NC_CACHE = {}


def kernel(x, w0, w1, w2):
    if "nc" not in _NC_CACHE:
        _NC_CACHE["nc"] = _build_program()
    nc = _NC_CACHE["nc"]
    s0, s1, s2 = _host_weights(w0, w1, w2)
    x = np.asarray(x, np.float32).reshape(64, H, W)
    in_maps = [
        {"x": np.ascontiguousarray(x[i * IMG_PER_CORE:(i + 1) * IMG_PER_CORE]),
         "s0": s0, "s1": s1, "s2": s2}
        for i in range(N_CORES)
    ]
    res = run_bass_kernel_spmd(nc, in_maps, list(range(N_CORES)))
    out = np.stack([np.asarray(res.results[i]["out"], np.float32)
                    for i in range(N_CORES)])
    return out.reshape(64, 1, H, W)


# revision 7
# speedup vs baseline: 2.5339x; 1.2158x over previous
"""Binary 3-layer CNN (sign activations + sign weights) on 8 NeuronCores.

Strategy: pure data parallel — 64 images -> 8 cores x 8 images.
Per core: 2 batches of 4 images; SBUF partition layout [128 = (4 img, 32 ch)].

This substrate has a large fixed cost per *instruction* (~0.1 ms) that is
nearly independent of instruction size, so the design minimizes instruction
count:
 - conv0: all 9 taps packed into the contraction dim (K=36 = 9 taps x 4
   images, via 9 tap-shifted DMA loads) -> ONE matmul per PSUM fill.
 - conv1/conv2: operands in fp8e4 (values +-1/0, exact) so tap-PAIRS pack
   into one DoubleRow matmul (2 K-tiles per instruction) -> 5 matmuls per
   fill instead of 9. rhs k-tile dim is a hand-built access pattern whose
   stride is the inter-tap offset (dy*WP + dx delta).
 - PSUM tiles span 4 banks [128, 2048]; each matmul writes one 512-f32 bank
   slice (ISA limit), one Sign activation evicts all 4 banks.
All matmul operands are exactly +-1/0 -> fp32 PSUM accumulation is exact.
Layers staged through padded DRAM scratch so conv padding is baked in.
"""

import numpy as np
import ml_dtypes

import concourse.bass as bass
import concourse.mybir as mybir
import concourse.tile as tile
from concourse import bacc
from concourse.bass_types import AP as RawAP
from concourse.bass_utils import run_bass_kernel_spmd

BF16 = mybir.dt.bfloat16
FP8 = mybir.dt.float8e4
F32 = mybir.dt.float32
F16 = mybir.dt.float16
AF = mybir.ActivationFunctionType
DR = mybir.MatmulPerfMode.DoubleRow

N_CORES = 8
IMG_PER_CORE = 8
B = 4          # images per partition-batch
H = W = 256
WP = 258       # padded width
HP = 258       # padded height
R = 64         # strip rows
NB = IMG_PER_CORE // B
TAPS = [(t // 3, t % 3) for t in range(9)]


def _conv_taps_dr(nc, ps_slice, s_t, hin, r):
    """9-tap conv into one 512-f32 psum bank slice: 4 DoubleRow + 1 single.
    hin is a padded fp8 tile [*, rows, WP]; r is the strip-local out row."""
    for pi, t0 in enumerate((0, 2, 4, 6)):
        dy0, dx0 = TAPS[t0]
        dy1, dx1 = TAPS[t0 + 1]
        delta = (dy1 - dy0) * WP + (dx1 - dx0)
        base = hin[:, r + dy0:r + dy0 + 2, dx0:dx0 + 256]
        rhs = RawAP(tensor=base.tensor,
                    ap=[[base.ap[0][0], 128], [delta, 2], [WP, 2], [1, 256]],
                    offset=base.offset)
        nc.tensor.matmul(ps_slice, s_t[:, t0:t0 + 2, :], rhs,
                         start=(pi == 0), stop=False, perf_mode=DR)
    nc.tensor.matmul(ps_slice, s_t[:, 8, :], hin[:, r + 2:r + 4, 2:WP],
                     start=False, stop=True)


def _build_program(stages=('0', 'A', 'B', 'C')):
    nc = bacc.Bacc("TRN2", target_bir_lowering=False, debug=False)

    x_in = nc.dram_tensor("x", [IMG_PER_CORE, H, W], F32, kind="ExternalInput")
    s0_in = nc.dram_tensor("s0", [36, 128], BF16, kind="ExternalInput")
    s1_in = nc.dram_tensor("s1", [128, 9, 128], FP8, kind="ExternalInput")
    s2_in = nc.dram_tensor("s2", [128, 9, 128], FP8, kind="ExternalInput")
    out_d = nc.dram_tensor("out", [IMG_PER_CORE, H, W], F16, kind="ExternalOutput")

    xs_d = nc.dram_tensor("xs", [IMG_PER_CORE, HP, WP], BF16)
    h0_d = nc.dram_tensor("h0", [NB, 128, HP, WP], FP8)
    h1_d = nc.dram_tensor("h1", [NB, 128, HP, WP], FP8)

    with tile.TileContext(nc) as tc:
        with (
            tc.tile_pool(name="const", bufs=1) as cpool,
            tc.tile_pool(name="xprep", bufs=2) as xpool,
            tc.tile_pool(name="a_in", bufs=1) as a_in_pool,
            tc.tile_pool(name="lay_in", bufs=2) as inpool,
            tc.tile_pool(name="lay_out", bufs=2) as outpool,
            tc.tile_pool(name="c_out", bufs=1) as cpool2,
            tc.tile_pool(name="psum", bufs=1, space="PSUM") as pspool,
        ):
            # --- constants ---
            s0t = cpool.tile([36, 128], BF16, tag="s0")
            nc.sync.dma_start(out=s0t[:, :], in_=s0_in[:, :])
            s1t = cpool.tile([128, 9, 128], FP8, tag="s1")
            nc.sync.dma_start(out=s1t[:, :, :], in_=s1_in[:, :, :])
            s2t = cpool.tile([128, 9, 128], FP8, tag="s2")
            nc.sync.dma_start(out=s2t[:, :, :], in_=s2_in[:, :, :])
            zt = cpool.tile([128, WP], FP8, tag="zt")
            nc.gpsimd.memset(zt[:, :], 0.0)
            ztw = cpool.tile([1, IMG_PER_CORE * WP], BF16, tag="ztw")
            nc.gpsimd.memset(ztw[:, :], 0.0)

            # --- pre-zero DRAM pad rows ---
            # xs: top and bottom padded row of every image, one DMA each
            for img in range(IMG_PER_CORE):
                nc.scalar.dma_start(out=xs_d[img, 0:1, :], in_=ztw[:, 0:WP])
                nc.scalar.dma_start(out=xs_d[img, HP - 1:HP, :],
                                    in_=ztw[:, 0:WP])
            for b in range(NB):
                for hd in (h0_d, h1_d):
                    nc.scalar.dma_start(out=hd[b, :, 0, :], in_=zt[:, :])
                    nc.scalar.dma_start(out=hd[b, :, HP - 1, :], in_=zt[:, :])

            # --- stage 0: sign(x) -> padded bf16 planes in DRAM, 4 img/tile ---
            if '0' in stages:
                for b in range(NB):
                    for rb in range(H // 128):
                        xf = xpool.tile([128, B, W], F32, tag="xf")
                        src = x_in[b * B:(b + 1) * B,
                                   rb * 128:(rb + 1) * 128, :]
                        nc.sync.dma_start(out=xf[:, :, :],
                                          in_=src.transpose([1, 0, 2]))
                        xp = xpool.tile([128, B, WP], BF16, tag="xp")
                        nc.vector.memset(xp[:, :, :], 0.0)
                        nc.scalar.activation(xp[:, :, 1:W + 1], xf[:, :, :],
                                             AF.Sign)
                        dst = xs_d[b * B:(b + 1) * B,
                                   rb * 128 + 1:(rb + 1) * 128 + 1, :]
                        nc.scalar.dma_start(out=dst.transpose([1, 0, 2]),
                                            in_=xp[:, :, :])

            for b in range(NB):
                # ---- stage A: conv0 (1 -> 32ch); K=36 (9 taps x 4 img) ----
                # x36[(t,g), r, c] = xs[g, 64s + r + dy_t, dx_t + c]
                if 'A' in stages:
                    for s in range(H // R):
                        x36 = a_in_pool.tile([36, R, W], BF16, tag="x36")
                        for t, (dy, dx) in enumerate(TAPS):
                            nc.sync.dma_start(
                                out=x36[B * t:B * (t + 1), :, :],
                                in_=xs_d[b * B:(b + 1) * B,
                                         s * R + dy:s * R + dy + R,
                                         dx:dx + 256])
                        ht = outpool.tile([128, R, WP], FP8, tag="ht")
                        nc.vector.memset(ht[:, :, :], 0.0)
                        for i in range(R // 16):
                            ps = pspool.tile([128, 4096], F32, tag="ps",
                                             name="ps")
                            for fi in range(8):
                                r = 16 * i + 2 * fi
                                nc.tensor.matmul(
                                    ps[:, fi * 512:(fi + 1) * 512],
                                    s0t[:, :], x36[:, r:r + 2, :],
                                    start=True, stop=True)
                            nc.scalar.activation(
                                ht[:, 16 * i:16 * i + 16, 1:W + 1],
                                ps[:, :].rearrange("p (a c) -> p a c", a=16),
                                AF.Sign)
                        nc.scalar.dma_start(
                            out=h0_d[b, :, s * R + 1:s * R + R + 1, :],
                            in_=ht[:, :, :])

                # ---- stage B: conv1 (32 -> 32ch); fp8 DoubleRow tap pairs ----
                if 'B' in stages:
                    for s in range(H // R):
                        hin = inpool.tile([128, R + 2, WP], FP8, tag="hin")
                        nc.sync.dma_start(
                            out=hin[:, :, :],
                            in_=h0_d[b, :, s * R:s * R + R + 2, :])
                        ht = outpool.tile([128, R, WP], FP8, tag="ht")
                        nc.vector.memset(ht[:, :, :], 0.0)
                        for i in range(R // 16):
                            ps = pspool.tile([128, 4096], F32, tag="ps",
                                             name="ps")
                            for fi in range(8):
                                _conv_taps_dr(
                                    nc, ps[:, fi * 512:(fi + 1) * 512],
                                    s1t, hin, 16 * i + 2 * fi)
                            nc.scalar.activation(
                                ht[:, 16 * i:16 * i + 16, 1:W + 1],
                                ps[:, :].rearrange("p (a c) -> p a c", a=16),
                                AF.Sign)
                        nc.scalar.dma_start(
                            out=h1_d[b, :, s * R + 1:s * R + R + 1, :],
                            in_=ht[:, :, :])

                # ---- stage C: conv2 (32 -> 1ch); M=4, fp8 DoubleRow ----
                if 'C' in stages:
                    for s in range(H // R):
                        hin = inpool.tile([128, R + 2, WP], FP8, tag="hin")
                        nc.sync.dma_start(
                            out=hin[:, :, :],
                            in_=h1_d[b, :, s * R:s * R + R + 2, :])
                        ot = cpool2.tile([B, R, W], F16, tag="ot")
                        for i in range(R // 16):
                            ps = pspool.tile([128, 4096], F32, tag="ps",
                                             name="ps")
                            for fi in range(8):
                                _conv_taps_dr(
                                    nc, ps[:, fi * 512:(fi + 1) * 512],
                                    s2t, hin, 16 * i + 2 * fi)
                            nc.vector.tensor_copy(
                                ot[:, 16 * i:16 * i + 16, :],
                                ps[0:B, :].rearrange("p (a c) -> p a c", a=16))
                        dst = out_d[b * B:(b + 1) * B, s * R:s * R + R, :]
                        nc.scalar.dma_start(out=dst, in_=ot[:, :, :])
    nc.compile()
    return nc


def _host_weights(w0, w1, w2):
    """Pack sign(w) into stationary matrices. tap index t = dy*3+dx."""
    w0s = np.sign(np.asarray(w0, np.float32))  # [32,1,3,3]
    w1s = np.sign(np.asarray(w1, np.float32))  # [32,32,3,3]
    w2s = np.sign(np.asarray(w2, np.float32))  # [1,32,3,3]
    s0 = np.zeros((36, 128), np.float32)
    s1 = np.zeros((128, 9, 128), np.float32)
    s2 = np.zeros((128, 9, 128), np.float32)
    for g in range(B):
        for t, (dy, dx) in enumerate(TAPS):
            # out[m=(g,co)] += s0[k=(t,g), m] * x36[k, pix]
            s0[t * B + g, g * 32:(g + 1) * 32] = w0s[:, 0, dy, dx]
            s1[g * 32:(g + 1) * 32, t, g * 32:(g + 1) * 32] = \
                w1s[:, :, dy, dx].T  # [ci, co]
            s2[g * 32:(g + 1) * 32, t, g] = w2s[0, :, dy, dx]
    return (s0.astype(ml_dtypes.bfloat16),
            s1.astype(ml_dtypes.float8_e4m3),
            s2.astype(ml_dtypes.float8_e4m3))


_NC_CACHE = {}


def kernel(x, w0, w1, w2):
    if "nc" not in _NC_CACHE:
        _NC_CACHE["nc"] = _build_program()
    nc = _NC_CACHE["nc"]
    s0, s1, s2 = _host_weights(w0, w1, w2)
    x = np.asarray(x, np.float32).reshape(64, H, W)
    in_maps = [
        {"x": np.ascontiguousarray(x[i * IMG_PER_CORE:(i + 1) * IMG_PER_CORE]),
         "s0": s0, "s1": s1, "s2": s2}
        for i in range(N_CORES)
    ]
    res = run_bass_kernel_spmd(nc, in_maps, list(range(N_CORES)))
    out = np.stack([np.asarray(res.results[i]["out"], np.float32)
                    for i in range(N_CORES)])
    return out.reshape(64, 1, H, W)
